# revision 30
# baseline (speedup 1.0000x reference)
"""LSTMCell Trainium2 kernel.

Full-input contract: kernel(**inputs) takes the complete (16384, 1024) fp32
tensors, shards the batch dim across 8 NeuronCores (data-parallel, weights
replicated), runs a Bass/Tile kernel per core, and gathers (h, c).

Per-core plan (B_local = 2048), measured ~490 us on-device vs a 444.7 us
pure-matmul floor (2048 MMs x 216 ns; PE ~89% busy):
  - Everything heavy is pre-packed on the HOST into fp16: weights are
    concatenated to [2048 (k), 4096 (gates i|f|o|u)] and kept
    SBUF-resident (128 KB/partition); x|h is transposed AND pre-tiled to
    [pair, partition, kt, 256] so each partition reads one contiguous
    8 KB run per 1 MB pair load (512 B runs measured ~7x slower); the
    bias arrives pre-broadcast as [128, 4096] (an on-device
    partition_broadcast put a ~20 us SBUF WAR hazard on the ramp).
    No on-device transposes or SWDGE casts remain.
  - Queue plan: ONE startup stream on the sync HWDGE queue in exact
    consumption order (xh0 k-quarters interleaved with the first weight
    column-half, then bias, then the second half, then pair-0 c and
    pair-1 xh); the scalar queue (later pairs' xh/c) stays empty early
    because a queue moving 4-8 KB descriptors starves any concurrent
    small-descriptor queue.
  - Ramp: streaming W by column halves makes needed k-tiles arrive every
    ~1.2 us while 8 PSUM banks cap per-k-tile matmul work at 1.7 us, so
    pair 0 sweeps BOTH m-tiles' first gate groups k-major and the ramp
    is PE-bound; the second groups run on resident weights.
  - Steady state per m-tile: two PSUM groups of 4 banks; each group
    accumulates 16 k-matmuls x 4 N=512 slices back-to-back, then DVE
    adds the bias (PSUM -> SBUF fp16) and ACT applies Sigmoid/Tanh
    while the other group's matmuls run.
  - Elementwise tail in fp16 on DVE/ACT: c' = f*c + i*u, h' = o*tanh(c');
    the last m-tile's drain is split into halves and its stores ride
    both queues to shorten the kernel tail.
  - h', c' are stored fp16 and upcast to fp32 on the host after gather.
"""

import sys

if "/opt/trn_rl_repo" not in sys.path:
    sys.path.insert(0, "/opt/trn_rl_repo")

import numpy as np

import concourse.bass as bass  # noqa: F401
import concourse.mybir as mybir
import concourse.tile as tile
from concourse import bacc
from concourse.bass_utils import run_bass_kernel_spmd

F32 = mybir.dt.float32
F16 = mybir.dt.float16

N_CORES = 8
B_FULL = 16384
IN = 1024
H = 1024
B_LOCAL = B_FULL // N_CORES  # 2048
P = 128
K_TILES = (IN + H) // P      # 16
N_TOTAL = 4 * H              # 4096 (gates i|f|o|u)
SIG = mybir.ActivationFunctionType.Sigmoid
TANH = mybir.ActivationFunctionType.Tanh
ADD = mybir.AluOpType.add
MULT = mybir.AluOpType.mult


class _NullCtx:
    def __enter__(self):
        return None

    def __exit__(self, *a):
        return False


def _maybe_for_i(tc, reps):
    return tc.For_i(0, reps, 1) if reps > 1 else _NullCtx()


def build_nc(b_local: int = B_LOCAL, reps: int = 1):
    """reps > 1 wraps the main loop in a For_i that recomputes the same
    outputs; usable for steady-state timing on hardware."""
    m_pairs = b_local // (2 * P)  # 8
    nc = bacc.Bacc("TRN2", target_bir_lowering=False, debug=False)

    # xh arrives host-pre-tiled as [pair, partition, kt, 256]: each partition
    # reads one contiguous 8 KB run per pair (512 B runs from a flat [K, M]
    # layout measured only ~60 GB/s vs ~430 GB/s for 8 KB descriptors).
    xh_d = nc.dram_tensor(
        "xh", [b_local // (2 * P), P, K_TILES, 2 * P], F16, kind="ExternalInput"
    )
    c_d = nc.dram_tensor("c", [b_local, H], F16, kind="ExternalInput")
    w_d = nc.dram_tensor("w", [IN + H, N_TOTAL], F16, kind="ExternalInput")
    # Bias arrives pre-broadcast from the host ([128, 4096] fp16): a plain
    # load beats an on-device partition_broadcast, whose SBUF staging tile
    # created a WAR hazard that stalled the first xh load ~20 us.
    b_d = nc.dram_tensor("b", [P, N_TOTAL], F16, kind="ExternalInput")
    ho_d = nc.dram_tensor("h_out", [b_local, H], F16, kind="ExternalOutput")
    co_d = nc.dram_tensor("c_out", [b_local, H], F16, kind="ExternalOutput")

    with tile.TileContext(nc) as tc:
        with (
            tc.tile_pool(name="wpool", bufs=1) as wpool,
            tc.tile_pool(name="const", bufs=1) as const,
        ):
            w16 = wpool.tile([P, K_TILES, N_TOTAL], F16)
            bb = const.tile([P, N_TOTAL], F16)

            with (
                tc.tile_pool(name="xh", bufs=2) as xhp,
                tc.tile_pool(name="cin", bufs=2) as cin,
                tc.tile_pool(name="gate", bufs=2) as gp,
                tc.tile_pool(name="tmp", bufs=2) as tp,
                tc.tile_pool(name="outp", bufs=2) as op,
                tc.tile_pool(name="ps", bufs=2, space="PSUM") as ps,
                _maybe_for_i(tc, reps),
            ):
                # Pair-0 activations go out on the sync queue AHEAD of the
                # weight stream: 1 MB at full rate (~3 us) so the first
                # matmul starts immediately. Later pairs ride the scalar
                # queue.
                # Single-queue startup stream in EXACT consumption order, so
                # the first matmul is gated by only 0.75 MB and weight
                # k-tiles then arrive every ~1.2 us (vs 1.7 us per-k-tile PE
                # work for pair 0's interleaved first-group sweeps):
                #   xh0[kt 0-3], W-colsA kt 0-3, xh0[4-7], W-colsA 4-7, ...
                #   then bias, then W-colsB (needed only after the first
                #   drain ~45 us), then pair-0 c, then pair-1 xh.
                # The scalar queue stays EMPTY early - a queue moving 4-8 KB
                # descriptors starves any concurrent small queue anyway.
                xh0 = xhp.tile([P, K_TILES, 2 * P], F16, tag="xh")
                colsA = slice(0, 2048)
                colsB = slice(2048, 4096)
                for q in range(4):
                    nc.sync.dma_start(
                        xh0[:, 4 * q : 4 * (q + 1), :],
                        xh_d.ap()[0][:, 4 * q : 4 * (q + 1), :],
                    )
                    for kt in range(4 * q, 4 * q + 4):
                        nc.sync.dma_start(
                            w16[:, kt, colsA], w_d.ap()[kt * P : (kt + 1) * P, colsA]
                        )
                nc.sync.dma_start(bb[:], b_d.ap())
                for kt in range(K_TILES):
                    nc.sync.dma_start(
                        w16[:, kt, colsB], w_d.ap()[kt * P : (kt + 1) * P, colsB]
                    )

                def mm_sweep(pt, lhs_m, slices, kt_range=range(K_TILES)):
                    for kt in kt_range:
                        for sj, s in enumerate(slices):
                            nc.tensor.matmul(
                                pt[:, sj * 512 : (sj + 1) * 512],
                                lhsT=lhs_m[:, kt, :],
                                rhs=w16[:, kt, s * 512 : (s + 1) * 512],
                                start=(kt == 0),
                                stop=(kt == K_TILES - 1),
                            )

                def drain(g, pt, gates, split=False):
                    gsl = slice(g * 2048, (g + 1) * 2048)
                    if g == 0:
                        # slices 0-3 = gates i,f -> sigmoid
                        nc.vector.tensor_tensor(gates[:, gsl], pt[:], bb[:, gsl], ADD)
                        nc.scalar.activation(gates[:, gsl], gates[:, gsl], SIG)
                    elif not split:
                        # slices 4,5 = gate o -> sigmoid; 6,7 = u -> tanh
                        nc.vector.tensor_tensor(gates[:, gsl], pt[:], bb[:, gsl], ADD)
                        nc.scalar.activation(
                            gates[:, 2048:3072], gates[:, 2048:3072], SIG
                        )
                        nc.scalar.activation(
                            gates[:, 3072:4096], gates[:, 3072:4096], TANH
                        )
                    else:
                        # Tail variant (last m-tile): h-half quarters so the
                        # first half's o and u finish first and its c'/h'
                        # chain starts while the second half still drains.
                        for h2 in range(2):
                            osl = slice(2048 + h2 * 512, 2048 + (h2 + 1) * 512)
                            usl = slice(3072 + h2 * 512, 3072 + (h2 + 1) * 512)
                            nc.vector.tensor_tensor(
                                gates[:, osl],
                                pt[:, h2 * 512 : (h2 + 1) * 512],
                                bb[:, osl], ADD,
                            )
                            nc.scalar.activation(gates[:, osl], gates[:, osl], SIG)
                            nc.vector.tensor_tensor(
                                gates[:, usl],
                                pt[:, 1024 + h2 * 512 : 1024 + (h2 + 1) * 512],
                                bb[:, usl], ADD,
                            )
                            nc.scalar.activation(gates[:, usl], gates[:, usl], TANH)

                def elementwise(m, gates, cprev, last=False):
                    rows = slice(m * P, (m + 1) * P)
                    t1 = tp.tile([P, H], F16, tag="t1")
                    t2 = tp.tile([P, H], F16, tag="t2")
                    t3 = tp.tile([P, H], F16, tag="t3")
                    cn = op.tile([P, H], F16, tag="cn")
                    hn = op.tile([P, H], F16, tag="hn")
                    # The last m-tile runs the chain in h-halves (pipelines
                    # DVE/ACT/stores, shortening the kernel tail ~3 us);
                    # elsewhere one full-width pass costs fewer instructions.
                    for hs in ([slice(h2 * 512, (h2 + 1) * 512) for h2 in range(2)]
                               if last else [slice(0, H)]):
                        i_h = gates[:, hs.start : hs.stop]
                        f_h = gates[:, H + hs.start : H + hs.stop]
                        o_h = gates[:, 2 * H + hs.start : 2 * H + hs.stop]
                        u_h = gates[:, 3 * H + hs.start : 3 * H + hs.stop]
                        nc.vector.tensor_tensor(t1[:, hs], f_h, cprev[:, hs], MULT)
                        nc.vector.tensor_tensor(t2[:, hs], i_h, u_h, MULT)
                        nc.vector.tensor_tensor(cn[:, hs], t1[:, hs], t2[:, hs], ADD)
                        nc.scalar.activation(t3[:, hs], cn[:, hs], TANH)
                        nc.vector.tensor_tensor(hn[:, hs], o_h, t3[:, hs], MULT)
                        # Stores split across both HWDGE queues on the tail.
                        nc.sync.dma_start(co_d.ap()[rows, hs], cn[:, hs])
                        (nc.scalar if last else nc.sync).dma_start(
                            ho_d.ap()[rows, hs], hn[:, hs]
                        )

                for pair in range(m_pairs):
                    if pair == 0:
                        xh = xh0
                    else:
                        # pair 1 rides sync right behind the weights (the
                        # scalar queue is descriptor-starved until the W
                        # stream drains); later pairs have enough slack.
                        xh = xhp.tile([P, K_TILES, 2 * P], F16, tag="xh")
                        (nc.sync if pair == 1 else nc.scalar).dma_start(
                            xh[:], xh_d.ap()[pair]
                        )
                    lhs0 = xh[:, :, 0:P]
                    lhs1 = xh[:, :, P : 2 * P]

                    # Pair-0 c loads ride sync (landing after the W stream,
                    # still well before the ~60 us elementwise) so they do
                    # not compete for HBM in the first microseconds.
                    c_eng = nc.sync if pair == 0 else nc.scalar
                    cprev0 = cin.tile([P, H], F16, tag="cprev")
                    c_eng.dma_start(
                        cprev0[:], c_d.ap()[pair * 2 * P : pair * 2 * P + P, :]
                    )
                    cprev1 = cin.tile([P, H], F16, tag="cprev")
                    c_eng.dma_start(
                        cprev1[:], c_d.ap()[pair * 2 * P + P : (pair + 1) * 2 * P, :]
                    )
                    gates0 = gp.tile([P, N_TOTAL], F16, tag="gates")
                    gates1 = gp.tile([P, N_TOTAL], F16, tag="gates")
                    m0 = pair * 2
                    m1 = pair * 2 + 1

                    if pair == 0:
                        # HAM warm-up: ~3.4 us of throwaway matmuls on the
                        # first landed xh chunk while the PE would idle
                        # waiting for the first weight tile. Opens the PE
                        # clock gate (4/8 -> 8/8) so the real stream starts
                        # warm (~2 us saved; results are overwritten by the
                        # start=True accumulations that follow).
                        warm = ps.tile([P, 4 * 512], F32, tag="pt")
                        for _ in range(8):
                            nc.tensor.matmul(
                                warm[:, 0:512],
                                lhsT=xh0[:, 0, 0:P],
                                rhs=xh0[:, 0:2, :],
                                start=True,
                                stop=True,
                            )
                        # Ramp: the weight k-tiles arrive every ~2.3 us and a
                        # PSUM bank takes one MM per k-tile, so sweep BOTH
                        # m-tiles' first gate groups k-major (8 MMs = 1.7 us
                        # per arriving k-tile) to track the weight stream.
                        ptA = ps.tile([P, 4 * 512], F32, tag="pt")
                        ptB = ps.tile([P, 4 * 512], F32, tag="pt")
                        for kt in range(K_TILES):
                            mm_sweep(ptA, lhs0, range(4), kt_range=[kt])
                            mm_sweep(ptB, lhs1, range(4), kt_range=[kt])
                        drain(0, ptA, gates0)
                        ptA2 = ps.tile([P, 4 * 512], F32, tag="pt")
                        mm_sweep(ptA2, lhs0, range(4, 8))
                        drain(0, ptB, gates1)
                        ptB2 = ps.tile([P, 4 * 512], F32, tag="pt")
                        mm_sweep(ptB2, lhs1, range(4, 8))
                        drain(1, ptA2, gates0)
                        elementwise(m0, gates0, cprev0)
                        drain(1, ptB2, gates1)
                        elementwise(m1, gates1, cprev1)
                    else:
                        last = pair == m_pairs - 1
                        for sub, (lhs_m, gates, cprev, m) in enumerate(
                            [(lhs0, gates0, cprev0, m0), (lhs1, gates1, cprev1, m1)]
                        ):
                            for g in range(2):
                                pt = ps.tile([P, 4 * 512], F32, tag="pt")
                                mm_sweep(pt, lhs_m, range(4 * g, 4 * g + 4))
                                drain(
                                    g, pt, gates, split=(last and sub == 1 and g == 1)
                                )
                            elementwise(m, gates, cprev, last=(last and sub == 1))

    nc.compile()
    return nc


_NC_CACHE: dict = {}


def _get_nc(b_local: int = B_LOCAL):
    if b_local not in _NC_CACHE:
        _NC_CACHE[b_local] = build_nc(b_local)
    return _NC_CACHE[b_local]


def make_in_maps(
    input, prev_h, prev_c,
    weight_xi, weight_hi, weight_xf, weight_hf,
    weight_xu, weight_hu, weight_xo, weight_ho,
    bias_i, bias_f, bias_o, bias_u,
):
    """Host-side shard/pack: batch split across cores, weights replicated.

    xhT per core: [x_core | h_core]^T as fp16, shape [IN+H, B_LOCAL]."""
    asnp = lambda a: np.asarray(a, dtype=np.float32)
    # Gate column order [i | f | o | u]; K rows: x-weights then h-weights.
    w_cat = np.concatenate(
        [
            np.concatenate([asnp(weight_xi), asnp(weight_xf), asnp(weight_xo), asnp(weight_xu)], axis=1),
            np.concatenate([asnp(weight_hi), asnp(weight_hf), asnp(weight_ho), asnp(weight_hu)], axis=1),
        ],
        axis=0,
    ).astype(np.float16)
    b_cat = np.concatenate([asnp(bias_i), asnp(bias_f), asnp(bias_o), asnp(bias_u)], axis=0)
    b_bcast = np.ascontiguousarray(
        np.broadcast_to(b_cat, (128, b_cat.shape[0]))
    ).astype(np.float16)
    xh16 = np.concatenate(
        [np.asarray(input), np.asarray(prev_h)], axis=1
    ).astype(np.float16)  # [B_FULL, IN+H]
    c16 = np.asarray(prev_c).astype(np.float16)
    in_maps = []
    n_pairs = B_LOCAL // (2 * P)
    for core in range(N_CORES):
        r = slice(core * B_LOCAL, (core + 1) * B_LOCAL)
        # [pair, p, kt, m2]: per (pair, partition) one contiguous 8 KB run.
        xh_pt = np.ascontiguousarray(
            xh16[r].reshape(n_pairs, 2 * P, K_TILES, P).transpose(0, 3, 2, 1)
        )
        in_maps.append(
            {
                "xh": xh_pt,
                "c": np.ascontiguousarray(c16[r]),
                "w": w_cat,
                "b": b_bcast,
            }
        )
    return in_maps


def kernel(**inputs):
    nc = _get_nc()
    in_maps = make_in_maps(**inputs)
    res = run_bass_kernel_spmd(nc, in_maps, core_ids=list(range(N_CORES)))
    h_full = np.concatenate(
        [res.results[c]["h_out"] for c in range(N_CORES)], axis=0
    ).astype(np.float32)
    c_full = np.concatenate(
        [res.results[c]["c_out"] for c in range(N_CORES)], axis=0
    ).astype(np.float32)
    return (h_full, c_full)


if __name__ == "__main__":
    rng = np.random.default_rng(0)
    stdv = 1.0 / np.sqrt(H)
    ins = {
        "input": rng.standard_normal((B_FULL, IN), dtype=np.float32),
        "prev_h": rng.standard_normal((B_FULL, H), dtype=np.float32),
        "prev_c": rng.standard_normal((B_FULL, H), dtype=np.float32),
    }
    for nm in ["weight_xi", "weight_hi", "weight_xf", "weight_hf",
               "weight_xu", "weight_hu", "weight_xo", "weight_ho"]:
        ins[nm] = rng.uniform(-stdv, stdv, (IN, H)).astype(np.float32)
    for nm in ["bias_i", "bias_f", "bias_o", "bias_u"]:
        ins[nm] = rng.uniform(-stdv, stdv, (H,)).astype(np.float32)
    h, c = kernel(**ins)
    print("kernel ran:", h.shape, c.shape)

    # quick host check against fp32 numpy reference
    def sig(x):
        return 1.0 / (1.0 + np.exp(-x))

    xi = ins["input"] @ ins["weight_xi"] + ins["prev_h"] @ ins["weight_hi"] + ins["bias_i"]
    xf = ins["input"] @ ins["weight_xf"] + ins["prev_h"] @ ins["weight_hf"] + ins["bias_f"]
    xo = ins["input"] @ ins["weight_xo"] + ins["prev_h"] @ ins["weight_ho"] + ins["bias_o"]
    xu = ins["input"] @ ins["weight_xu"] + ins["prev_h"] @ ins["weight_hu"] + ins["bias_u"]
    cr = sig(xf) * ins["prev_c"] + sig(xi) * np.tanh(xu)
    hr = sig(xo) * np.tanh(cr)
    print("h err:", np.abs(h - hr).max(), "c err:", np.abs(c - cr).max())


# revision 32
# speedup vs baseline: 1.0036x; 1.0036x over previous
"""LSTMCell Trainium2 kernel.

Full-input contract: kernel(**inputs) takes the complete (16384, 1024) fp32
tensors, shards the batch dim across 8 NeuronCores (data-parallel, weights
replicated), runs a Bass/Tile kernel per core, and gathers (h, c).

Per-core plan (B_local = 2048), measured ~490 us on-device vs a 444.7 us
pure-matmul floor (2048 MMs x 216 ns; PE ~89% busy):
  - Everything heavy is pre-packed on the HOST into fp16: weights are
    concatenated to [2048 (k), 4096 (gates i|f|o|u)] and kept
    SBUF-resident (128 KB/partition); x|h is transposed AND pre-tiled to
    [pair, partition, kt, 256] so each partition reads one contiguous
    8 KB run per 1 MB pair load (512 B runs measured ~7x slower); the
    bias arrives pre-broadcast as [128, 4096] (an on-device
    partition_broadcast put a ~20 us SBUF WAR hazard on the ramp).
    No on-device transposes or SWDGE casts remain.
  - Queue plan: ONE startup stream on the sync HWDGE queue in exact
    consumption order (xh0 k-quarters interleaved with the first weight
    column-half, then bias, then the second half, then pair-0 c and
    pair-1 xh); the scalar queue (later pairs' xh/c) stays empty early
    because a queue moving 4-8 KB descriptors starves any concurrent
    small-descriptor queue.
  - Ramp: streaming W by column halves makes needed k-tiles arrive every
    ~1.2 us while 8 PSUM banks cap per-k-tile matmul work at 1.7 us, so
    pair 0 sweeps BOTH m-tiles' first gate groups k-major and the ramp
    is PE-bound; the second groups run on resident weights.
  - Steady state per m-tile: two PSUM groups of 4 banks; each group
    accumulates 16 k-matmuls x 4 N=512 slices back-to-back, then DVE
    adds the bias (PSUM -> SBUF fp16) and ACT applies Sigmoid/Tanh
    while the other group's matmuls run.
  - Elementwise tail in fp16 on DVE/ACT: c' = f*c + i*u, h' = o*tanh(c');
    the last m-tile's drain is split into halves and its stores ride
    both queues to shorten the kernel tail.
  - h', c' are stored fp16 and upcast to fp32 on the host after gather.
"""

import sys

if "/opt/trn_rl_repo" not in sys.path:
    sys.path.insert(0, "/opt/trn_rl_repo")

import numpy as np

import concourse.bass as bass  # noqa: F401
import concourse.mybir as mybir
import concourse.tile as tile
from concourse import bacc
from concourse.bass_utils import run_bass_kernel_spmd

F32 = mybir.dt.float32
F16 = mybir.dt.float16

N_CORES = 8
B_FULL = 16384
IN = 1024
H = 1024
B_LOCAL = B_FULL // N_CORES  # 2048
P = 128
K_TILES = (IN + H) // P      # 16
N_TOTAL = 4 * H              # 4096 (gates i|f|o|u)
SIG = mybir.ActivationFunctionType.Sigmoid
TANH = mybir.ActivationFunctionType.Tanh
ADD = mybir.AluOpType.add
MULT = mybir.AluOpType.mult


class _NullCtx:
    def __enter__(self):
        return None

    def __exit__(self, *a):
        return False


def _maybe_for_i(tc, reps):
    return tc.For_i(0, reps, 1) if reps > 1 else _NullCtx()


def build_nc(b_local: int = B_LOCAL, reps: int = 1):
    """reps > 1 wraps the main loop in a For_i that recomputes the same
    outputs; usable for steady-state timing on hardware."""
    m_pairs = b_local // (2 * P)  # 8
    nc = bacc.Bacc("TRN2", target_bir_lowering=False, debug=False)

    # xh arrives host-pre-tiled as [pair, partition, kt, 256]: each partition
    # reads one contiguous 8 KB run per pair (512 B runs from a flat [K, M]
    # layout measured only ~60 GB/s vs ~430 GB/s for 8 KB descriptors).
    xh_d = nc.dram_tensor(
        "xh", [b_local // (2 * P), P, K_TILES, 2 * P], F16, kind="ExternalInput"
    )
    c_d = nc.dram_tensor("c", [b_local, H], F16, kind="ExternalInput")
    w_d = nc.dram_tensor("w", [IN + H, N_TOTAL], F16, kind="ExternalInput")
    # Bias arrives pre-broadcast from the host ([128, 4096] fp16): a plain
    # load beats an on-device partition_broadcast, whose SBUF staging tile
    # created a WAR hazard that stalled the first xh load ~20 us.
    b_d = nc.dram_tensor("b", [P, N_TOTAL], F16, kind="ExternalInput")
    ho_d = nc.dram_tensor("h_out", [b_local, H], F16, kind="ExternalOutput")
    co_d = nc.dram_tensor("c_out", [b_local, H], F16, kind="ExternalOutput")

    with tile.TileContext(nc) as tc:
        with (
            tc.tile_pool(name="wpool", bufs=1) as wpool,
            tc.tile_pool(name="const", bufs=1) as const,
        ):
            w16 = wpool.tile([P, K_TILES, N_TOTAL], F16)
            bb = const.tile([P, N_TOTAL], F16)

            with (
                tc.tile_pool(name="xh", bufs=2) as xhp,
                tc.tile_pool(name="cin", bufs=2) as cin,
                tc.tile_pool(name="gate", bufs=2) as gp,
                tc.tile_pool(name="tmp", bufs=2) as tp,
                tc.tile_pool(name="outp", bufs=2) as op,
                tc.tile_pool(name="ps", bufs=2, space="PSUM") as ps,
                _maybe_for_i(tc, reps),
            ):
                # Pair-0 activations go out on the sync queue AHEAD of the
                # weight stream: 1 MB at full rate (~3 us) so the first
                # matmul starts immediately. Later pairs ride the scalar
                # queue.
                # Single-queue startup stream in EXACT consumption order, so
                # the first matmul is gated by only 0.75 MB and weight
                # k-tiles then arrive every ~1.2 us (vs 1.7 us per-k-tile PE
                # work for pair 0's interleaved first-group sweeps):
                #   xh0[kt 0-3], W-colsA kt 0-3, xh0[4-7], W-colsA 4-7, ...
                #   then bias, then W-colsB (needed only after the first
                #   drain ~45 us), then pair-0 c, then pair-1 xh.
                # The scalar queue stays EMPTY early - a queue moving 4-8 KB
                # descriptors starves any concurrent small queue anyway.
                xh0 = xhp.tile([P, K_TILES, 2 * P], F16, tag="xh")
                colsA = slice(0, 2048)
                colsB = slice(2048, 4096)
                # Finest pieces first: the first real matmul is gated by
                # only xh0[kt0] (64 KB) + W[kt0, first 512 cols] (128 KB).
                nc.sync.dma_start(xh0[:, 0:1, :], xh_d.ap()[0][:, 0:1, :])
                nc.sync.dma_start(w16[:, 0, 0:512], w_d.ap()[0:P, 0:512])
                nc.sync.dma_start(w16[:, 0, 512:2048], w_d.ap()[0:P, 512:2048])
                nc.sync.dma_start(xh0[:, 1:4, :], xh_d.ap()[0][:, 1:4, :])
                for kt in range(1, 4):
                    nc.sync.dma_start(
                        w16[:, kt, colsA], w_d.ap()[kt * P : (kt + 1) * P, colsA]
                    )
                for q in range(1, 4):
                    nc.sync.dma_start(
                        xh0[:, 4 * q : 4 * (q + 1), :],
                        xh_d.ap()[0][:, 4 * q : 4 * (q + 1), :],
                    )
                    for kt in range(4 * q, 4 * q + 4):
                        nc.sync.dma_start(
                            w16[:, kt, colsA], w_d.ap()[kt * P : (kt + 1) * P, colsA]
                        )
                nc.sync.dma_start(bb[:], b_d.ap())
                for kt in range(K_TILES):
                    nc.sync.dma_start(
                        w16[:, kt, colsB], w_d.ap()[kt * P : (kt + 1) * P, colsB]
                    )

                # HAM warm-up on a memset tile: ~3.4 us of throwaway matmuls
                # starting as soon as the engines come up, while the first
                # data is still in flight. The PE is then continuously busy
                # into the real stream, so the clock gate opens (4/8 -> 8/8)
                # before the first real matmul instead of ~3.4 us after it.
                warm_t = tp.tile([P, 5 * P], F16, tag="t1")
                nc.vector.memset(warm_t[:], 0.0)
                warm = ps.tile([P, 4 * 512], F32, tag="pt")
                for _ in range(8):
                    nc.tensor.matmul(
                        warm[:, 0:512],
                        lhsT=warm_t[:, 0:P],
                        rhs=warm_t[:, P : 5 * P],
                        start=True,
                        stop=True,
                    )

                def mm_sweep(pt, lhs_m, slices, kt_range=range(K_TILES)):
                    for kt in kt_range:
                        for sj, s in enumerate(slices):
                            nc.tensor.matmul(
                                pt[:, sj * 512 : (sj + 1) * 512],
                                lhsT=lhs_m[:, kt, :],
                                rhs=w16[:, kt, s * 512 : (s + 1) * 512],
                                start=(kt == 0),
                                stop=(kt == K_TILES - 1),
                            )

                def drain(g, pt, gates, split=False):
                    gsl = slice(g * 2048, (g + 1) * 2048)
                    if g == 0:
                        # slices 0-3 = gates i,f -> sigmoid
                        nc.vector.tensor_tensor(gates[:, gsl], pt[:], bb[:, gsl], ADD)
                        nc.scalar.activation(gates[:, gsl], gates[:, gsl], SIG)
                    elif not split:
                        # slices 4,5 = gate o -> sigmoid; 6,7 = u -> tanh
                        nc.vector.tensor_tensor(gates[:, gsl], pt[:], bb[:, gsl], ADD)
                        nc.scalar.activation(
                            gates[:, 2048:3072], gates[:, 2048:3072], SIG
                        )
                        nc.scalar.activation(
                            gates[:, 3072:4096], gates[:, 3072:4096], TANH
                        )
                    else:
                        # Tail variant (last m-tile): h-half quarters so the
                        # first half's o and u finish first and its c'/h'
                        # chain starts while the second half still drains.
                        for h2 in range(2):
                            osl = slice(2048 + h2 * 512, 2048 + (h2 + 1) * 512)
                            usl = slice(3072 + h2 * 512, 3072 + (h2 + 1) * 512)
                            nc.vector.tensor_tensor(
                                gates[:, osl],
                                pt[:, h2 * 512 : (h2 + 1) * 512],
                                bb[:, osl], ADD,
                            )
                            nc.scalar.activation(gates[:, osl], gates[:, osl], SIG)
                            nc.vector.tensor_tensor(
                                gates[:, usl],
                                pt[:, 1024 + h2 * 512 : 1024 + (h2 + 1) * 512],
                                bb[:, usl], ADD,
                            )
                            nc.scalar.activation(gates[:, usl], gates[:, usl], TANH)

                def elementwise(m, gates, cprev, last=False):
                    rows = slice(m * P, (m + 1) * P)
                    t1 = tp.tile([P, H], F16, tag="t1")
                    t2 = tp.tile([P, H], F16, tag="t2")
                    t3 = tp.tile([P, H], F16, tag="t3")
                    cn = op.tile([P, H], F16, tag="cn")
                    hn = op.tile([P, H], F16, tag="hn")
                    # The last m-tile runs the chain in h-halves (pipelines
                    # DVE/ACT/stores, shortening the kernel tail ~3 us);
                    # elsewhere one full-width pass costs fewer instructions.
                    for hs in ([slice(h2 * 512, (h2 + 1) * 512) for h2 in range(2)]
                               if last else [slice(0, H)]):
                        i_h = gates[:, hs.start : hs.stop]
                        f_h = gates[:, H + hs.start : H + hs.stop]
                        o_h = gates[:, 2 * H + hs.start : 2 * H + hs.stop]
                        u_h = gates[:, 3 * H + hs.start : 3 * H + hs.stop]
                        nc.vector.tensor_tensor(t1[:, hs], f_h, cprev[:, hs], MULT)
                        nc.vector.tensor_tensor(t2[:, hs], i_h, u_h, MULT)
                        nc.vector.tensor_tensor(cn[:, hs], t1[:, hs], t2[:, hs], ADD)
                        nc.scalar.activation(t3[:, hs], cn[:, hs], TANH)
                        nc.vector.tensor_tensor(hn[:, hs], o_h, t3[:, hs], MULT)
                        # Stores split across both HWDGE queues on the tail.
                        nc.sync.dma_start(co_d.ap()[rows, hs], cn[:, hs])
                        (nc.scalar if last else nc.sync).dma_start(
                            ho_d.ap()[rows, hs], hn[:, hs]
                        )

                for pair in range(m_pairs):
                    if pair == 0:
                        xh = xh0
                    else:
                        # pair 1 rides sync right behind the weights (the
                        # scalar queue is descriptor-starved until the W
                        # stream drains); later pairs have enough slack.
                        xh = xhp.tile([P, K_TILES, 2 * P], F16, tag="xh")
                        (nc.sync if pair == 1 else nc.scalar).dma_start(
                            xh[:], xh_d.ap()[pair]
                        )
                    lhs0 = xh[:, :, 0:P]
                    lhs1 = xh[:, :, P : 2 * P]

                    # Pair-0 c loads ride sync (landing after the W stream,
                    # still well before the ~60 us elementwise) so they do
                    # not compete for HBM in the first microseconds.
                    c_eng = nc.sync if pair == 0 else nc.scalar
                    cprev0 = cin.tile([P, H], F16, tag="cprev")
                    c_eng.dma_start(
                        cprev0[:], c_d.ap()[pair * 2 * P : pair * 2 * P + P, :]
                    )
                    cprev1 = cin.tile([P, H], F16, tag="cprev")
                    c_eng.dma_start(
                        cprev1[:], c_d.ap()[pair * 2 * P + P : (pair + 1) * 2 * P, :]
                    )
                    gates0 = gp.tile([P, N_TOTAL], F16, tag="gates")
                    gates1 = gp.tile([P, N_TOTAL], F16, tag="gates")
                    m0 = pair * 2
                    m1 = pair * 2 + 1

                    if pair == 0:
                        # Ramp: the weight k-tiles arrive every ~2.3 us and a
                        # PSUM bank takes one MM per k-tile, so sweep BOTH
                        # m-tiles' first gate groups k-major (8 MMs = 1.7 us
                        # per arriving k-tile) to track the weight stream.
                        ptA = ps.tile([P, 4 * 512], F32, tag="pt")
                        ptB = ps.tile([P, 4 * 512], F32, tag="pt")
                        for kt in range(K_TILES):
                            mm_sweep(ptA, lhs0, range(4), kt_range=[kt])
                            mm_sweep(ptB, lhs1, range(4), kt_range=[kt])
                        drain(0, ptA, gates0)
                        ptA2 = ps.tile([P, 4 * 512], F32, tag="pt")
                        mm_sweep(ptA2, lhs0, range(4, 8))
                        drain(0, ptB, gates1)
                        ptB2 = ps.tile([P, 4 * 512], F32, tag="pt")
                        mm_sweep(ptB2, lhs1, range(4, 8))
                        drain(1, ptA2, gates0)
                        elementwise(m0, gates0, cprev0)
                        drain(1, ptB2, gates1)
                        elementwise(m1, gates1, cprev1)
                    else:
                        last = pair == m_pairs - 1
                        for sub, (lhs_m, gates, cprev, m) in enumerate(
                            [(lhs0, gates0, cprev0, m0), (lhs1, gates1, cprev1, m1)]
                        ):
                            for g in range(2):
                                pt = ps.tile([P, 4 * 512], F32, tag="pt")
                                mm_sweep(pt, lhs_m, range(4 * g, 4 * g + 4))
                                drain(
                                    g, pt, gates, split=(last and sub == 1 and g == 1)
                                )
                            elementwise(m, gates, cprev, last=(last and sub == 1))

    nc.compile()
    return nc


_NC_CACHE: dict = {}


def _get_nc(b_local: int = B_LOCAL):
    if b_local not in _NC_CACHE:
        _NC_CACHE[b_local] = build_nc(b_local)
    return _NC_CACHE[b_local]


def make_in_maps(
    input, prev_h, prev_c,
    weight_xi, weight_hi, weight_xf, weight_hf,
    weight_xu, weight_hu, weight_xo, weight_ho,
    bias_i, bias_f, bias_o, bias_u,
):
    """Host-side shard/pack: batch split across cores, weights replicated.

    xhT per core: [x_core | h_core]^T as fp16, shape [IN+H, B_LOCAL]."""
    asnp = lambda a: np.asarray(a, dtype=np.float32)
    # Gate column order [i | f | o | u]; K rows: x-weights then h-weights.
    w_cat = np.concatenate(
        [
            np.concatenate([asnp(weight_xi), asnp(weight_xf), asnp(weight_xo), asnp(weight_xu)], axis=1),
            np.concatenate([asnp(weight_hi), asnp(weight_hf), asnp(weight_ho), asnp(weight_hu)], axis=1),
        ],
        axis=0,
    ).astype(np.float16)
    b_cat = np.concatenate([asnp(bias_i), asnp(bias_f), asnp(bias_o), asnp(bias_u)], axis=0)
    b_bcast = np.ascontiguousarray(
        np.broadcast_to(b_cat, (128, b_cat.shape[0]))
    ).astype(np.float16)
    xh16 = np.concatenate(
        [np.asarray(input), np.asarray(prev_h)], axis=1
    ).astype(np.float16)  # [B_FULL, IN+H]
    c16 = np.asarray(prev_c).astype(np.float16)
    in_maps = []
    n_pairs = B_LOCAL // (2 * P)
    for core in range(N_CORES):
        r = slice(core * B_LOCAL, (core + 1) * B_LOCAL)
        # [pair, p, kt, m2]: per (pair, partition) one contiguous 8 KB run.
        xh_pt = np.ascontiguousarray(
            xh16[r].reshape(n_pairs, 2 * P, K_TILES, P).transpose(0, 3, 2, 1)
        )
        in_maps.append(
            {
                "xh": xh_pt,
                "c": np.ascontiguousarray(c16[r]),
                "w": w_cat,
                "b": b_bcast,
            }
        )
    return in_maps


def kernel(**inputs):
    nc = _get_nc()
    in_maps = make_in_maps(**inputs)
    res = run_bass_kernel_spmd(nc, in_maps, core_ids=list(range(N_CORES)))
    h_full = np.concatenate(
        [res.results[c]["h_out"] for c in range(N_CORES)], axis=0
    ).astype(np.float32)
    c_full = np.concatenate(
        [res.results[c]["c_out"] for c in range(N_CORES)], axis=0
    ).astype(np.float32)
    return (h_full, c_full)


if __name__ == "__main__":
    rng = np.random.default_rng(0)
    stdv = 1.0 / np.sqrt(H)
    ins = {
        "input": rng.standard_normal((B_FULL, IN), dtype=np.float32),
        "prev_h": rng.standard_normal((B_FULL, H), dtype=np.float32),
        "prev_c": rng.standard_normal((B_FULL, H), dtype=np.float32),
    }
    for nm in ["weight_xi", "weight_hi", "weight_xf", "weight_hf",
               "weight_xu", "weight_hu", "weight_xo", "weight_ho"]:
        ins[nm] = rng.uniform(-stdv, stdv, (IN, H)).astype(np.float32)
    for nm in ["bias_i", "bias_f", "bias_o", "bias_u"]:
        ins[nm] = rng.uniform(-stdv, stdv, (H,)).astype(np.float32)
    h, c = kernel(**ins)
    print("kernel ran:", h.shape, c.shape)

    # quick host check against fp32 numpy reference
    def sig(x):
        return 1.0 / (1.0 + np.exp(-x))

    xi = ins["input"] @ ins["weight_xi"] + ins["prev_h"] @ ins["weight_hi"] + ins["bias_i"]
    xf = ins["input"] @ ins["weight_xf"] + ins["prev_h"] @ ins["weight_hf"] + ins["bias_f"]
    xo = ins["input"] @ ins["weight_xo"] + ins["prev_h"] @ ins["weight_ho"] + ins["bias_o"]
    xu = ins["input"] @ ins["weight_xu"] + ins["prev_h"] @ ins["weight_hu"] + ins["bias_u"]
    cr = sig(xf) * ins["prev_c"] + sig(xi) * np.tanh(xu)
    hr = sig(xo) * np.tanh(cr)
    print("h err:", np.abs(h - hr).max(), "c err:", np.abs(c - cr).max())
